# revision 5
# baseline (speedup 1.0000x reference)
"""Bahdanau attention kernel for 8 Trainium2 NeuronCores.

Math note: in the reference,
    score = (tanh(enc @ Wh + bh) + (dec @ Ws + bs)[:, None, :]) @ Wv + bv
    attn  = softmax(score, axis=T)
the decoder projection term and bv are constant across T, and softmax is
shift-invariant along T — so the decoder branch cancels exactly and both
outputs depend only on softmax_T(tanh(enc @ Wh + bh) @ Wv) and enc itself.

Sharding: data-parallel over batch (8 batches per core). Per core:
  - big matmul enclinT[u, bt] = Wh[h,u]^T-tiles (stationary) x encT[h, bt]
    (moving) in bf16, accumulated fp32 in PSUM
  - tanh (+bh per-partition bias) on ScalarE -> bf16 tiles
  - score[1, bt] = Wv^T-tile (stationary) x tanh tiles, accumulated in PSUM
  - per-batch softmax on [1, 512] (max / exp+accum / reciprocal)
  - context via VectorE tensor_tensor_reduce in fp32: attn row broadcast
    across partitions (DMA), multiplied with fp32 encT tiles, reduced along
    the free (t) dim -> context^T columns
Host pre-computes encT (fp32 + bf16 copies) per core; outputs are gathered
and context^T is rearranged on host.
"""

from contextlib import ExitStack

import numpy as np
import ml_dtypes

import concourse.bass as bass
import concourse.tile as tile
from concourse import bacc, mybir
from concourse.bass_utils import run_bass_kernel_spmd

B, T, H, U = 64, 512, 1024, 1024
NCORES = 8
BL = B // NCORES          # 8 batches per core
P = 128
NH = H // P               # 8 h-tiles
NU = U // P               # 8 u-tiles

f32 = mybir.dt.float32
bf16 = mybir.dt.bfloat16
AF = mybir.ActivationFunctionType
ALU = mybir.AluOpType
BF16 = ml_dtypes.bfloat16


def build_kernel_body(tc, aps):
    nc = tc.nc
    enc_bf_d = aps["enc_bf"]      # [H, BL*T] bf16
    enc_f32_d = aps["enc_f32"]    # [H, BL*T] f32
    wh_d = aps["wh"]              # [H, U] bf16
    wv_d = aps["wv"]              # [P, NU] bf16  (column j = Wv[j*128:(j+1)*128])
    bh_d = aps["bh"]              # [P, NU] f32   (column j = bh[j*128:(j+1)*128])
    attn_d = aps["attn"]          # [BL, T] f32 out
    ctxt_d = aps["ctxt"]          # [NH, P, BL] f32 out (context^T tiles)

    ctx = aps["_ctx"]
    cpool = ctx.enter_context(tc.tile_pool(name="const", bufs=1))
    ebf_pool = ctx.enter_context(tc.tile_pool(name="ebf", bufs=2))
    ef_pool = ctx.enter_context(tc.tile_pool(name="ef", bufs=2))
    th_pool = ctx.enter_context(tc.tile_pool(name="th", bufs=2))
    bc_pool = ctx.enter_context(tc.tile_pool(name="bc", bufs=2))
    scr_pool = ctx.enter_context(tc.tile_pool(name="scr", bufs=2))
    row_pool = ctx.enter_context(tc.tile_pool(name="row", bufs=2))
    ctx_pool = ctx.enter_context(tc.tile_pool(name="ctxt", bufs=1))
    pe_pool = ctx.enter_context(tc.tile_pool(name="pe", bufs=3, space="PSUM"))
    sc_pool = ctx.enter_context(tc.tile_pool(name="sc", bufs=2, space="PSUM"))

    # persistent constants
    wh_t = []
    for i in range(NH):
        t_ = cpool.tile([P, U], bf16, tag=f"wh{i}")
        nc.sync.dma_start(t_[:], wh_d[i * P:(i + 1) * P, :])
        wh_t.append(t_)
    wv_sb = cpool.tile([P, NU], bf16, tag="wv")
    nc.sync.dma_start(wv_sb[:], wv_d[:, :])
    bh_sb = cpool.tile([P, NU], f32, tag="bh")
    nc.sync.dma_start(bh_sb[:], bh_d[:, :])

    ctx_t = []
    for h in range(NH):
        t_ = ctx_pool.tile([P, BL], f32, tag=f"ctx{h}")
        ctx_t.append(t_)

    for b in range(BL):
        # stream this batch's encT tiles
        ebf = []
        for h in range(NH):
            t_ = ebf_pool.tile([P, T], bf16, tag=f"ebf{h}")
            nc.sync.dma_start(t_[:], enc_bf_d[h * P:(h + 1) * P, b * T:(b + 1) * T])
            ebf.append(t_)
        ef = []
        for h in range(NH):
            t_ = ef_pool.tile([P, T], f32, tag=f"ef{h}")
            nc.sync.dma_start(t_[:], enc_f32_d[h * P:(h + 1) * P, b * T:(b + 1) * T])
            ef.append(t_)

        # big matmul + tanh per u-tile
        tanh_ts = []
        for j in range(NU):
            pe = pe_pool.tile([P, T], f32, tag="pe")
            for h in range(NH):
                nc.tensor.matmul(
                    pe[:],
                    wh_t[h][:, j * P:(j + 1) * P],
                    ebf[h][:],
                    start=(h == 0),
                    stop=(h == NH - 1),
                )
            th = th_pool.tile([P, T], bf16, tag=f"th{j}")
            nc.scalar.activation(th[:], pe[:], AF.Tanh, bias=bh_sb[:, j:j + 1])
            tanh_ts.append(th)

        # score[1, T] = sum_u Wv[u] * tanh[u, t]
        ps = sc_pool.tile([1, T], f32, tag="score")
        for j in range(NU):
            nc.tensor.matmul(
                ps[:],
                wv_sb[:, j:j + 1],
                tanh_ts[j][:],
                start=(j == 0),
                stop=(j == NU - 1),
                skip_group_check=True,
            )

        # softmax over T on partition 0
        nmax = row_pool.tile([1, 1], f32, tag="nmax")
        nc.vector.tensor_reduce(nmax[:], ps[:], axis=mybir.AxisListType.X,
                                op=ALU.max, negate=True)
        erow = row_pool.tile([1, T], f32, tag="erow")
        ssum = row_pool.tile([1, 1], f32, tag="ssum")
        nc.scalar.activation(erow[:], ps[:], AF.Exp, bias=nmax[:],
                             accum_out=ssum[:])
        rrec = row_pool.tile([1, 1], f32, tag="rrec")
        nc.vector.reciprocal(rrec[:], ssum[:])
        arow = row_pool.tile([1, T], f32, tag="arow")
        nc.vector.tensor_scalar_mul(arow[:], erow[:], rrec[:])
        nc.sync.dma_start(attn_d[b:b + 1, :], arow[:])

        # broadcast attn row across partitions, then fp32 weighted reduce
        bc = bc_pool.tile([P, T], f32, tag="bc")
        nc.gpsimd.partition_broadcast(bc[:], arow[:])
        for h in range(NH):
            scr = scr_pool.tile([P, T], f32, tag="scr")
            nc.vector.tensor_mul(scr[:], ef[h][:], bc[:])
            nc.vector.tensor_reduce(ctx_t[h][:, b:b + 1], scr[:],
                                    axis=mybir.AxisListType.X, op=ALU.add)

    for h in range(NH):
        nc.sync.dma_start(ctxt_d[h], ctx_t[h][:])


def build_nc():
    nc = bacc.Bacc("TRN2", target_bir_lowering=False, debug=False,
                   num_devices=NCORES)
    aps = {
        "enc_bf": nc.dram_tensor("enc_bf", [H, BL * T], bf16,
                                 kind="ExternalInput").ap(),
        "enc_f32": nc.dram_tensor("enc_f32", [H, BL * T], f32,
                                  kind="ExternalInput").ap(),
        "wh": nc.dram_tensor("wh", [H, U], bf16, kind="ExternalInput").ap(),
        "wv": nc.dram_tensor("wv", [P, NU], bf16, kind="ExternalInput").ap(),
        "bh": nc.dram_tensor("bh", [P, NU], f32, kind="ExternalInput").ap(),
        "attn": nc.dram_tensor("attn", [BL, T], f32,
                               kind="ExternalOutput").ap(),
        "ctxt": nc.dram_tensor("ctxt", [NH, P, BL], f32,
                               kind="ExternalOutput").ap(),
    }
    with tile.TileContext(nc) as tc:
        with ExitStack() as body_ctx:
            aps["_ctx"] = body_ctx
            build_kernel_body(tc, aps)
    nc.compile()
    return nc


def make_in_maps(enc_output, Wh, bh, Wv):
    enc = np.ascontiguousarray(np.asarray(enc_output, dtype=np.float32))
    wh_bf = np.asarray(Wh, dtype=np.float32).astype(BF16)
    wv_t = np.ascontiguousarray(
        np.asarray(Wv, dtype=np.float32).reshape(NU, P).T).astype(BF16)
    bh_t = np.ascontiguousarray(
        np.asarray(bh, dtype=np.float32).reshape(NU, P).T)
    in_maps = []
    for c in range(NCORES):
        shard = enc[c * BL:(c + 1) * BL].reshape(BL * T, H)
        encT = np.ascontiguousarray(shard.T)          # [H, BL*T] f32
        in_maps.append({
            "enc_bf": encT.astype(BF16),
            "enc_f32": encT,
            "wh": wh_bf,
            "wv": wv_t,
            "bh": bh_t,
        })
    return in_maps


_NC_CACHE = None


def kernel(dec_hidden, enc_output, Wh, bh, Ws, bs, Wv, bv, **_unused):
    global _NC_CACHE
    if _NC_CACHE is None:
        _NC_CACHE = build_nc()
    nc = _NC_CACHE
    in_maps = make_in_maps(enc_output, Wh, bh, Wv)
    res = run_bass_kernel_spmd(nc, in_maps, list(range(NCORES))).results
    attn = np.concatenate([res[c]["attn"] for c in range(NCORES)], axis=0)
    ctx_parts = []
    for c in range(NCORES):
        ct = res[c]["ctxt"]                          # [NH, P, BL]
        ctx_parts.append(np.ascontiguousarray(
            ct.transpose(2, 0, 1).reshape(BL, H)))   # [BL, H]
    context = np.concatenate(ctx_parts, axis=0)
    return context.astype(np.float32), attn.astype(np.float32)
